# revision 37
# baseline (speedup 1.0000x reference)
"""Trainium2 Bass kernel for nn_Attention_27719718929033 (v3).

Channel-attention block + 3x3 conv, data-parallel over batch across 8 cores.

v3 changes vs v2 (217.8us, and a latent xpad read-before-write race):
  * X^T tiles for the gram matmul come from xbar DMA-transposes of the fp16
    x16 copy (SBUF->SBUF on the ACT HWDGE queue) instead of 128 PE
    fp32r transpose matmuls + fp32 staging + PSUM drain copies.  Removes
    ~7us of PE work, ~10us of DVE/ScalarE drain work, and one full fp32
    HBM read of x.
  * attention-add runs inside the DMA datapath: the attn token-major
    rows are DMA-accumulated (CCE add) onto the winograd inverse-transform
    output in SBUF, then stored with one more DMA.  Removes the separate
    GpSimd/DVE add ops, the ar staging tiles, and the fp32 co tiles.
  * out is declared fp16 in DRAM and upcast on host (exact): halves the
    output write traffic.
  * xpad border memsets + interior DMAs get explicit dependency edges to
    the first DVE op of each v_build: the v2 flat-view reads raced with
    the strided border memsets (nondeterministic NaN / 2e-2 errors).
  * o2->attn DRAM writes stay on sync; acc-DMAs get explicit deps on the
    o2 writes they read (cross-queue DRAM RAW).

Math (per batch, X = x[b] in [C, N] fp16, N = 4096):
    G = X X^T (gram); logits = Wq_s G Wk^T; A = softmax_rows(logits)
    M = pw A Wv; out2 = X^T M^T  (token-major [N, C])
    reference reshape [N, C] -> [B, C, H, W] is a flat reinterpretation =>
    write out2 token-major to DRAM fp16, DMA-accumulate slices of it onto
    the conv output (channel-major read absorbs the reinterpretation).
    conv = F(4,3)-Winograd 3x3 along H; fp16 everywhere, fp32 PSUM.
"""
from contextlib import ExitStack

import numpy as np

import concourse.bacc as bacc
import concourse.mybir as mybir
import concourse.tile as tile
from concourse.bass_utils import run_bass_kernel_spmd
from concourse.tile_rust import add_dep_helper

N_CORES = 8
B, C, H, W = 16, 256, 64, 64
BL = B // N_CORES  # batches per core
N = H * W  # tokens
HP = H + 2  # padded height
WP = W + 2  # padded width
CK = C // 128  # channel chunks of 128
TT = N // 128  # token tiles of 128
TY = H // 4  # winograd tiles along H (4 output rows each)
NHB = H // 32  # conv h-blocks of 32 rows
SCALE = C ** (-0.5)

F32 = mybir.dt.float32
F32R = mybir.dt.float32r
F16 = mybir.dt.float16

# F(4,3) Winograd weight transform (Lavin), for correlation (= lax conv).
G_WINO = np.array(
    [
        [1 / 4, 0, 0],
        [-1 / 6, -1 / 6, -1 / 6],
        [-1 / 6, 1 / 6, -1 / 6],
        [1 / 24, 1 / 12, 1 / 6],
        [1 / 24, -1 / 12, 1 / 6],
        [0, 0, 1],
    ],
    dtype=np.float64,
)


def build_program(use_qkv_bias, use_v_bias, use_proj_bias, use_conv_bias):
    nc = bacc.Bacc(None, target_bir_lowering=False)
    AL = mybir.AluOpType

    x = nc.declare_dram_parameter("x", [BL, C, N], F32R, isOutput=False)
    wqk_t = nc.declare_dram_parameter("wqk_t", [C, 2 * C], F16, isOutput=False)
    wv = nc.declare_dram_parameter("wv", [C, C], F16, isOutput=False)
    pw_t = nc.declare_dram_parameter("pw_t", [C, C], F16, isOutput=False)
    # gw[kc][ic(128), p(6), dy(3), oc(256)] fp16 winograd-transformed conv w
    gw = nc.declare_dram_parameter("gw", [CK, 128, 6, 3, C], F16, isOutput=False)
    bqk = bv = pb = cb = None
    if use_qkv_bias:
        bqk = nc.declare_dram_parameter("bqk", [2 * C], F16, isOutput=False)
    if use_v_bias:
        bv = nc.declare_dram_parameter("bv", [C], F16, isOutput=False)
    if use_proj_bias:
        pb = nc.declare_dram_parameter("pb", [C], F16, isOutput=False)
    if use_conv_bias:
        cb = nc.declare_dram_parameter("cb", [C], F32, isOutput=False)
    ident32 = nc.declare_dram_parameter("ident32", [128, 128], F32R, isOutput=False)
    out = nc.declare_dram_parameter("out", [BL, C, N], F16, isOutput=True)

    attn_dram = nc.dram_tensor("attn_scratch", [BL, N, C], F16)

    with tile.TileContext(nc) as tc, ExitStack() as ctx:
        # --- persistent SBUF pools ---
        xs_pool = ctx.enter_context(tc.tile_pool(name="x16", bufs=1))
        xp_pool = ctx.enter_context(tc.tile_pool(name="xpad", bufs=4))
        v_pool = ctx.enter_context(tc.tile_pool(name="vwin", bufs=2))
        w_pool = ctx.enter_context(tc.tile_pool(name="weights", bufs=1))
        sm_pool = ctx.enter_context(tc.tile_pool(name="smx", bufs=2))
        attn_pool = ctx.enter_context(tc.tile_pool(name="attnmat", bufs=1))
        o2_pool = ctx.enter_context(tc.tile_pool(name="o2", bufs=3))
        m_pool = ctx.enter_context(tc.tile_pool(name="msb", bufs=3))
        it_pool = ctx.enter_context(tc.tile_pool(name="invtmp", bufs=1))
        ar_pool = ctx.enter_context(tc.tile_pool(name="attnrd", bufs=2))
        # PSUM: 2 (g/logits) + 2 (mm rotation) + 4 (conv point-pairs) = 8 banks
        at_ps_pool = ctx.enter_context(
            tc.tile_pool(name="atps", bufs=2, space="PSUM")
        )
        mm_ps_pool = ctx.enter_context(
            tc.tile_pool(name="mmps", bufs=2, space="PSUM")
        )
        cv_ps_pool = ctx.enter_context(
            tc.tile_pool(name="cvps", bufs=2, space="PSUM")
        )

        # --- weights to SBUF ---
        wqk_sb = w_pool.tile([128, CK, 2 * C], F16, tag="wqk")
        wv_sb = w_pool.tile([128, CK, C], F16, tag="wv")
        pw_sb = w_pool.tile([128, CK, C], F16, tag="pw")
        gw_sb = [
            w_pool.tile([128, 6, 3, C], F16, tag=f"gw{kc}", name=f"gw_sb{kc}")
            for kc in range(CK)
        ]

        def weights_in():
            # emitted after the b0 staging DMAs: the sync queue serves the
            # head-critical staging first (wqk first needed by T1 ~18us)
            for kc in range(CK):
                nc.sync.dma_start(
                    wqk_sb[:, kc, :], wqk_t[kc * 128 : (kc + 1) * 128, :]
                )
            for kc in range(CK):
                nc.sync.dma_start(wv_sb[:, kc, :], wv[kc * 128 : (kc + 1) * 128, :])
                nc.sync.dma_start(pw_sb[:, kc, :], pw_t[kc * 128 : (kc + 1) * 128, :])
            for kc in range(CK):
                nc.sync.dma_start(gw_sb[kc][:], gw[kc])

        ones1 = None
        if use_qkv_bias or use_v_bias or use_proj_bias:
            ones1 = w_pool.tile([1, 128], F16, tag="ones")
            nc.gpsimd.memset(ones1[:], 1.0)
        bqk_sb = None
        if use_qkv_bias:
            bqk_sb = w_pool.tile([1, 2 * C], F16, tag="bqk")
            nc.sync.dma_start(bqk_sb[:], bqk[:].rearrange("c -> 1 c"))
        bv_sb = None
        if use_v_bias:
            bv_sb = w_pool.tile([128, CK], F16, tag="bv")
            for dc in range(CK):
                nc.sync.dma_start(
                    bv_sb[:, dc], bv[dc * 128 : (dc + 1) * 128].rearrange("p -> p 1")
                )
        pb_sb = None
        if use_proj_bias:
            pb_sb = w_pool.tile([1, C], F16, tag="pb")
            nc.sync.dma_start(pb_sb[:], pb[:].rearrange("c -> 1 c"))
        cb_sb = None
        if use_conv_bias:
            cb_sb = w_pool.tile([128, CK], F32, tag="cb")
            for oc in range(CK):
                nc.sync.dma_start(
                    cb_sb[:, oc], cb[oc * 128 : (oc + 1) * 128].rearrange("p -> p 1")
                )

        # --- input: fp32r staging chunks (sync HWDGE, line rate).  x16 via
        # ScalarE casts from staging; X^T tiles via PE fp32r transpose-mode
        # matmuls inside g_phase (xbar DMA-transposes act as global DMA
        # barriers in Tile and serialized the whole head).
        x16 = [
            [
                xs_pool.tile([128, N], F16, tag=f"x{b}{ck}", name=f"x16_{b}_{ck}")
                for ck in range(CK)
            ]
            for b in range(BL)
        ]
        ident32_sb = w_pool.tile([128, 128], F32R, tag="ident32")
        nc.sync.dma_start(ident32_sb[:], ident32[:])
        stage_pool = ctx.enter_context(tc.tile_pool(name="xstage", bufs=6))
        NSC = 1024  # staging chunk tokens
        stage = {}
        x16_casts = {}

        def x_load(b):
            for s in range(4):
                for ck in range(CK):
                    o = s * NSC
                    st = stage_pool.tile([128, NSC], F32R, tag="xstg")
                    nc.sync.dma_start(
                        st[:], x[b, ck * 128 : (ck + 1) * 128, o : o + NSC]
                    )
                    ci = nc.scalar.copy(
                        x16[b][ck][:, o : o + NSC], st[:].bitcast(F32)
                    )
                    x16_casts.setdefault((b, ck), []).append(ci)
                    stage[(b, ck, s)] = st

        def tok_window(b, ck, t):
            # stationary [128 chan, 128 tokens] fp16 (one contiguous free dim)
            return x16[b][ck][:, t * 128 : (t + 1) * 128]

        # --- xr: padded, row-deinterleaved fp16 x for the winograd V build ---
        # layout [128, 4, 18, WP]: image row h lives at [h % 4, h // 4 + 1];
        # top pad (h=-1) at [3][0], bottom pad (h=64) at [0][17].  Strided
        # casting DMAs from DRAM (gpsimd SWDGE): ~1.4us gen + ~1.2us
        # transfer each, and no engine time.
        xpad = {}
        xpad_deps = {}  # (b, kc) -> list of producing insts (for v_build deps)

        def xpad_in(b, kc, from_dram=False):
            xp = xp_pool.tile(
                [128, 4, 18, WP], F16, tag="xp", name=f"xpad{b}_{kc}"
            )
            # borders: pad cols 0/65 for all rows; pad rows at flat (r m)
            # indices [0][17] = 17 and [3][0] = 54 (stride 37).
            m1 = nc.gpsimd.memset(
                xp[:].rearrange("p r m w -> p (r m) w")[:, :, 0 :: WP - 1], 0.0
            )
            m2 = nc.gpsimd.memset(
                xp[:].rearrange("p r m w -> p (r m) w")[:, 17:55:37, :], 0.0
            )
            deps = [m1, m2]
            if from_dram:
                # b0: strided casting DMAs straight from DRAM (SWDGE) --
                # they run before x16 exists and overlap the staging reads
                xv = x[b, kc * 128 : (kc + 1) * 128, :].bitcast(F32).rearrange(
                    "p (h w) -> p h w", w=W
                )
                for r in range(4):
                    deps.append(
                        nc.gpsimd.dma_start(
                            xp[:, r, 1:17, 1 : W + 1], xv[:, r::4, :]
                        )
                    )
            else:
                # b1: plain SBUF->SBUF copies from x16 on the sync HWDGE
                # queue: no HBM traffic, no SWDGE-queue blockage ahead of
                # the conv acc DMAs
                xv = x16[b][kc][:].rearrange("p (h w) -> p h w", w=W)
                for r in range(4):
                    d = nc.sync.dma_start(
                        xp[:, r, 1:17, 1 : W + 1], xv[:, r::4, :]
                    )
                    for ci in x16_casts[(b, kc)]:
                        add_dep_helper(
                            d.ins, ci.ins, sync=True, reason="x16 cast done"
                        )
                    deps.append(d)
            xpad[(b, kc)] = xp
            xpad_deps[(b, kc)] = deps

        v_sb = {}

        def v_build(b, kc):
            # V_p = sum_q BT[p,q] d_q with d_q = deinterleaved xpad phase
            # reads (flat contiguous [128, 1056] -> DVE 2x TT / 4x TS modes).
            vt = v_pool.tile(
                [128, 6, TY, WP], F16, tag=f"v{b}", name=f"v_{b}_{kc}"
            )
            vtf = vt[:].rearrange("p s t w -> p (s t w)")
            xpf = xpad[(b, kc)][:].rearrange("p r m w -> p (r m w)")
            SEG = TY * WP  # 1056

            def d(q):
                # d(q)[ty] = padded row 4ty+q: [r=(q-1)%4][slot (q-1)//4+1..]
                r, s0 = (q - 1) % 4, (q - 1) // 4 + 1
                off = r * (18 * WP) + s0 * WP
                return xpf[:, off : off + SEG]

            def vs(j):
                return vtf[:, j * SEG : (j + 1) * SEG]

            V = nc.vector
            TS, TTo = V.tensor_scalar_mul, V.tensor_tensor
            # V0 = 4 d0 - 5 d2 + d4   (v2 as scratch)
            first = TS(vs(2), d(2), -5.0)
            for dep in xpad_deps[(b, kc)]:
                add_dep_helper(
                    first.ins, dep.ins, sync=True, reason="xpad ready"
                )
            TS(vs(0), d(0), 4.0)
            TTo(vs(0), vs(0), d(4), op=AL.add)
            TTo(vs(0), vs(0), vs(2), op=AL.add)
            # V5 = 4 d1 - 5 d3 + d5   (v1 as scratch)
            TS(vs(1), d(1), 4.0)
            TTo(vs(1), vs(1), d(5), op=AL.add)
            TS(vs(5), d(3), -5.0)
            TTo(vs(5), vs(5), vs(1), op=AL.add)
            # V1 = -4(d1 + d2) + (d3 + d4)   (v2 as scratch)
            TTo(vs(2), d(1), d(2), op=AL.add)
            TTo(vs(1), d(3), d(4), op=AL.add)
            TS(vs(2), vs(2), -4.0)
            TTo(vs(1), vs(1), vs(2), op=AL.add)
            # V2 = 4(d1 - d2) + (d4 - d3)    (v3 as scratch)
            TTo(vs(2), d(1), d(2), op=AL.subtract)
            TS(vs(2), vs(2), 4.0)
            TTo(vs(3), d(4), d(3), op=AL.subtract)
            TTo(vs(2), vs(2), vs(3), op=AL.add)
            # V3 = 2u + v, V4 = -2u + v with u = d3-d1, v = d4-d2
            TTo(vs(4), d(3), d(1), op=AL.subtract)
            TTo(vs(3), d(4), d(2), op=AL.subtract)
            TS(vs(4), vs(4), 2.0)
            TTo(vs(3), vs(3), vs(4), op=AL.add)
            TS(vs(4), vs(4), -2.0)
            TTo(vs(4), vs(3), vs(4), op=AL.add)
            v_sb[(b, kc)] = vt

        # ---------------- attention (gram path, fp16 operands) ----------------
        lg_pss = {}
        a_sbs = {}

        def qk_phase(b):
            # explicit fused [Q|K] + logits (only used when qkv bias nonzero)
            lg_ps = at_ps_pool.tile([128, CK, C], F32, tag="atps", name=f"lg_ps{b}")
            for t in range(TT):
                qk_ps = mm_ps_pool.tile([128, 2 * C], F32, tag="qkps")
                for kc in range(CK):
                    nc.tensor.matmul(
                        qk_ps[:],
                        tok_window(b, kc, t),
                        wqk_sb[:, kc, :],
                        start=(kc == 0),
                        stop=(kc == CK - 1 and not use_qkv_bias),
                    )
                if use_qkv_bias:
                    nc.tensor.matmul(
                        qk_ps[:], ones1[:], bqk_sb[:], start=False, stop=True
                    )
                qk_sb = sm_pool.tile([128, 2 * C], F16, tag="qksb")
                nc.vector.tensor_copy(qk_sb[:], qk_ps[:])

                for cc in range(CK):
                    mm = nc.tensor.matmul(
                        lg_ps[:, cc, :],
                        qk_sb[:, cc * 128 : (cc + 1) * 128],
                        qk_sb[:, C : 2 * C],
                        start=(t == 0 and cc == 0),
                        stop=(t == TT - 1),
                        skip_group_check=True,
                    )
                    if t == 0 and cc == 0:
                        lg_clear = mm
                    elif t == 0:
                        add_dep_helper(
                            mm.ins, lg_clear.ins, sync=False,
                            reason="after lg bank clear",
                        )
            lg_pss[b] = lg_ps

        def g_phase(b):
            # logits = Wq_s (X X^T) Wk^T; X^T tiles via fp32r transpose-mode
            # matmuls on the staging tiles (~73ns each), drained fp16
            g_ps = at_ps_pool.tile([128, CK, C], F32, tag="atps", name=f"g_ps{b}")
            g_clear = None
            for t2 in range(TT // 2):
                xt_ps = mm_ps_pool.tile([128, 2, C], F32, tag="qkps")
                tclear = None
                for j in range(2):
                    t = 2 * t2 + j
                    s, jj = divmod(t, NSC // 128)
                    for ck in range(CK):
                        mm = nc.tensor.matmul(
                            xt_ps[:, j, ck * 128 : (ck + 1) * 128].bitcast(F32R),
                            stage[(b, ck, s)][:, jj * 128 : (jj + 1) * 128],
                            ident32_sb[:],
                            is_transpose=True,
                            start=(j == 0 and ck == 0),
                            stop=(j == 1 and ck == CK - 1),
                            skip_group_check=True,
                        )
                        if j == 0 and ck == 0:
                            tclear = mm
                        else:
                            add_dep_helper(
                                mm.ins, tclear.ins, sync=False,
                                reason="after xt bank clear",
                            )
                xt_sb = sm_pool.tile([128, 2, C], F16, tag="qksb", bufs=4)
                if t2 % 2 == 0:
                    nc.vector.tensor_copy(xt_sb[:], xt_ps[:])
                else:
                    nc.scalar.copy(xt_sb[:], xt_ps[:])
                for j in range(2):
                    t = 2 * t2 + j
                    for cc in range(CK):
                        mm = nc.tensor.matmul(
                            g_ps[:, cc, :],
                            xt_sb[:, j, cc * 128 : (cc + 1) * 128],
                            xt_sb[:, j, :],
                            start=(t == 0 and cc == 0),
                            stop=(t == TT - 1),
                            skip_group_check=True,
                        )
                        if t == 0 and cc == 0:
                            g_clear = mm
                        elif t == 0:
                            add_dep_helper(
                                mm.ins, g_clear.ins, sync=False,
                                reason="after g bank clear",
                            )
            g_sb = attn_pool.tile([128, CK, C], F16, tag="g", name=f"g_sb{b}")
            nc.scalar.copy(g_sb[:, 0, :], g_ps[:, 0, :])
            nc.scalar.copy(g_sb[:, 1, :], g_ps[:, 1, :])

            # T1 = G Wk^T
            t1_ps = mm_ps_pool.tile([128, CK, C], F32, tag="qkps", name=f"t1_ps{b}")
            t1_clear = None
            for cpc in range(CK):
                for dc in range(CK):
                    mm = nc.tensor.matmul(
                        t1_ps[:, cpc, :],
                        g_sb[:, dc, cpc * 128 : (cpc + 1) * 128],
                        wqk_sb[:, dc, C : 2 * C],
                        start=(cpc == 0 and dc == 0),
                        stop=(dc == CK - 1),
                        skip_group_check=True,
                    )
                    if cpc == 0 and dc == 0:
                        t1_clear = mm
                    elif dc == 0:
                        add_dep_helper(
                            mm.ins, t1_clear.ins, sync=False,
                            reason="after t1 bank clear",
                        )
            t1_sb = attn_pool.tile([128, CK, C], F16, tag="t1", name=f"t1_sb{b}")
            nc.scalar.copy(t1_sb[:, 0, :], t1_ps[:, 0, :])
            nc.scalar.copy(t1_sb[:, 1, :], t1_ps[:, 1, :])

            # logits = Wq_s T1
            lg_ps = at_ps_pool.tile([128, CK, C], F32, tag="atps", name=f"glg_ps{b}")
            lg_clear = None
            for cc in range(CK):
                for kc in range(CK):
                    mm = nc.tensor.matmul(
                        lg_ps[:, cc, :],
                        wqk_sb[:, kc, cc * 128 : (cc + 1) * 128],
                        t1_sb[:, kc, :],
                        start=(cc == 0 and kc == 0),
                        stop=(kc == CK - 1),
                        skip_group_check=True,
                    )
                    if cc == 0 and kc == 0:
                        lg_clear = mm
                    elif kc == 0:
                        add_dep_helper(
                            mm.ins, lg_clear.ins, sync=False,
                            reason="after glg bank clear",
                        )
            lg_pss[b] = lg_ps

        def softmax_phase(b):
            lg_ps = lg_pss[b]
            a_sb = attn_pool.tile([128, CK, C], F16, tag="a", name=f"a_sb{b}")
            ex = sm_pool.tile([128, CK, C], F16, tag="ex")
            for cc in range(CK):
                nmx = sm_pool.tile([128, 1], F32, tag=f"nmx{cc}", name=f"nmx{b}_{cc}")
                nc.vector.reduce_max(
                    nmx[:], lg_ps[:, cc, :], axis=mybir.AxisListType.X, negate=True
                )
                sm = sm_pool.tile([128, 1], F32, tag=f"sm{cc}", name=f"sm{b}_{cc}")
                nc.scalar.activation(
                    ex[:, cc, :],
                    lg_ps[:, cc, :],
                    mybir.ActivationFunctionType.Exp,
                    bias=nmx[:],
                    scale=1.0,
                    accum_out=sm[:],
                )
                rs = sm_pool.tile([128, 1], F32, tag=f"rs{cc}", name=f"rs{b}_{cc}")
                nc.vector.reciprocal(rs[:], sm[:])
                nc.vector.tensor_scalar_mul(a_sb[:, cc, :], ex[:, cc, :], rs[:])
            a_sbs[b] = a_sb

        o2_dmas = {}  # (b, oc-half) -> list of o2 write DMAs

        def rest_phase(b):
            a_sb = a_sbs[b]
            # U = A^T P^T
            u_sb = attn_pool.tile([128, CK, C], F16, tag="u", name=f"u_sb{b}")
            u_ps = mm_ps_pool.tile([128, CK, C], F32, tag="qkps", name=f"u_ps{b}")
            for dc in range(CK):
                for cc in range(CK):
                    mm = nc.tensor.matmul(
                        u_ps[:, dc, :],
                        a_sb[:, cc, dc * 128 : (dc + 1) * 128],
                        pw_sb[:, cc, :],
                        start=(dc == 0 and cc == 0),
                        stop=(cc == CK - 1),
                        skip_group_check=True,
                    )
                    if dc == 0 and cc == 0:
                        u_clear = mm
                    elif cc == 0:
                        add_dep_helper(
                            mm.ins, u_clear.ins, sync=False,
                            reason="after u bank clear",
                        )
            nc.scalar.copy(u_sb[:, 0, :], u_ps[:, 0, :])
            nc.scalar.copy(u_sb[:, 1, :], u_ps[:, 1, :])

            # M^T = Wv^T U
            mt_sb = attn_pool.tile([128, CK, C], F16, tag="mt", name=f"mt_sb{b}")
            mt_ps = mm_ps_pool.tile([128, CK, C], F32, tag="qkps", name=f"mt_ps{b}")
            for cpc in range(CK):
                for dc in range(CK):
                    mm = nc.tensor.matmul(
                        mt_ps[:, cpc, :],
                        wv_sb[:, dc, cpc * 128 : (cpc + 1) * 128],
                        u_sb[:, dc, :],
                        start=(cpc == 0 and dc == 0),
                        stop=(dc == CK - 1),
                        skip_group_check=True,
                    )
                    if cpc == 0 and dc == 0:
                        mt_clear = mm
                    elif dc == 0:
                        add_dep_helper(
                            mm.ins, mt_clear.ins, sync=False,
                            reason="after mt bank clear",
                        )
            nc.scalar.copy(mt_sb[:, 0, :], mt_ps[:, 0, :])
            nc.scalar.copy(mt_sb[:, 1, :], mt_ps[:, 1, :])

            # r^T = bv^T U + pb
            use_r = use_v_bias or use_proj_bias
            r_sb = None
            if use_r:
                r_ps = mm_ps_pool.tile([1, C], F32, tag="qkps")
                started = False
                if use_v_bias:
                    for dc in range(CK):
                        nc.tensor.matmul(
                            r_ps[:],
                            bv_sb[:, dc],
                            u_sb[:, dc, :],
                            start=(dc == 0),
                            stop=(dc == CK - 1 and not use_proj_bias),
                        )
                    started = True
                if use_proj_bias:
                    nc.tensor.matmul(
                        r_ps[:],
                        ones1[0:1, 0:1],
                        pb_sb[:],
                        start=not started,
                        stop=True,
                    )
                r_sb = attn_pool.tile([1, C], F16, tag="r", name=f"r_sb{b}")
                nc.vector.tensor_copy(r_sb[:], r_ps[:])

            # out2[n, e] = sum_c' X[c', n] M^T[c', e] (+ 1 r^T), fp16 to DRAM
            # two token-tiles share one PSUM bank + one drain copy + one DMA
            o2_dmas[(b, 0)] = []
            o2_dmas[(b, 1)] = []
            for t in range(0, TT, 2):
                o_ps = mm_ps_pool.tile([128, 2, C], F32, tag="qkps")
                o_clear = None
                for j in range(2):
                    for kc in range(CK):
                        mm = nc.tensor.matmul(
                            o_ps[:, j, :],
                            tok_window(b, kc, t + j),
                            mt_sb[:, kc, :],
                            start=(j == 0 and kc == 0),
                            stop=(kc == CK - 1 and not use_r),
                            skip_group_check=True,
                        )
                        if j == 0 and kc == 0:
                            o_clear = mm
                        elif kc == 0:
                            add_dep_helper(
                                mm.ins, o_clear.ins, sync=False,
                                reason="after o2 bank clear",
                            )
                    if use_r:
                        nc.tensor.matmul(
                            o_ps[:, j, :], ones1[:], r_sb[:], start=False, stop=True
                        )
                o_sb = o2_pool.tile([128, 2, C], F16, tag="o2sb")
                nc.scalar.copy(o_sb[:], o_ps[:])
                dma = nc.sync.dma_start(
                    attn_dram[b, t * 128 : (t + 2) * 128, :].rearrange(
                        "(a p) c -> p a c", p=128
                    ),
                    o_sb[:],
                )
                o2_dmas[(b, t // 16)].append(dma)

        # ---------------- conv: winograd point matmuls + inverse ----------------
        # tiles are (oc, ty0, nty): nty*4 output rows each.  The final tiles
        # run as halves (nty=4) to shorten the drain->inverse->acc->store
        # tail after the last matmul.
        ALL_TILES = [(oc, hb * 8, 8) for oc in range(CK) for hb in range(NHB)]

        def conv_phase(b, tiles=None, late=False):
            attn_chw = attn_dram[b].rearrange("(p q) c -> p q c", p=C)
            for oc, ty0, nty in tiles if tiles is not None else ALL_TILES:
                FD = nty * 64  # moving free dim per point matmul
                ar = None
                if late:
                    # prefetch the attn rows; the add runs on DVE so the
                    # tail skips the ~5us accumulate-DMA latency
                    ar = ar_pool.tile([128, nty, C], F16, tag="ar")
                    ard = nc.sync.dma_start(
                        ar[:],
                        attn_chw[oc * 128 : (oc + 1) * 128, ty0 : ty0 + nty, :],
                    )
                    for dep in o2_dmas[(b, oc)]:
                        add_dep_helper(
                            ard.ins, dep.ins, sync=True,
                            reason="attn rows written",
                        )
                m_sb = m_pool.tile(
                    [128, 6, FD], F16, tag="m", name=f"m_{b}_{oc}_{ty0}"
                )
                for grp in range(3):  # point pairs (0,1),(2,3),(4,5)
                    mp = cv_ps_pool.tile([128, 2, FD], F32, tag="cvps")
                    for pp in range(2):
                        p = grp * 2 + pp
                        for dx in range(3):
                            for kc in range(CK):
                                nc.tensor.matmul(
                                    mp[:, pp, :],
                                    gw_sb[kc][
                                        :, p, dx, oc * 128 : (oc + 1) * 128
                                    ],
                                    v_sb[(b, kc)][
                                        :, p, ty0 : ty0 + nty, dx : dx + W
                                    ],
                                    start=(dx == 0 and kc == 0),
                                    stop=(dx == 2 and kc == CK - 1),
                                )
                    if use_conv_bias and grp == 0:
                        # fold conv bias into m0 only: A^T row sums give
                        # y_i += cb exactly for i=0 and 0 elsewhere
                        nc.scalar.activation(
                            m_sb[:, 0, :], mp[:, 0, :],
                            mybir.ActivationFunctionType.Copy,
                            bias=cb_sb[:, oc], scale=1.0,
                        )
                        nc.scalar.copy(m_sb[:, 1, :], mp[:, 1, :])
                    else:
                        nc.scalar.copy(m_sb[:, 2 * grp : 2 * grp + 2, :], mp[:])

                # A^T inverse transform (DVE); y-phase outputs land
                # interleaved (rows i::4) in the dead m1..m4 slots so the
                # flat [128, 2048] view is row-major [32, 64]
                it = it_pool.tile([128, 6, FD], F16, tag="it")
                m_ = [m_sb[:, p, :] for p in range(6)]
                ia, ib, ic_, id_, ie, it3 = (it[:, j] for j in range(6))
                y16 = m_sb[:, 1:5, :].rearrange(
                    "p a f -> p (a f)"
                ).rearrange("p (h w) -> p h w", w=W)

                def rv(ap):
                    return ap.rearrange("p (ty w) -> p ty w", w=W)

                V = nc.vector
                V.tensor_tensor(ia, m_[1], m_[2], op=AL.subtract)
                V.tensor_tensor(ib, m_[3], m_[4], op=AL.subtract)
                V.tensor_tensor(ic_, m_[1], m_[2], op=AL.add)
                V.tensor_tensor(id_, m_[3], m_[4], op=AL.add)
                # m1..m4 are dead from here; their slots hold y0..y3 phases
                V.tensor_tensor(ie, ic_, id_, op=AL.add)
                V.tensor_tensor(y16[:, 0::4, :], rv(ie), rv(m_[0]), op=AL.add)
                V.scalar_tensor_tensor(it3, ib, 8.0, m_[5], AL.mult, AL.add)
                V.tensor_tensor(y16[:, 3::4, :], rv(it3), rv(ia), op=AL.add)
                V.scalar_tensor_tensor(
                    y16[:, 1::4, :], rv(ib), 2.0, rv(ia), AL.mult, AL.add
                )
                last = V.scalar_tensor_tensor(
                    y16[:, 2::4, :], rv(id_), 4.0, rv(ic_), AL.mult, AL.add
                )
                yflat = m_sb[:, 1:5, :].rearrange("p a f -> p (a f)")
                if late:
                    # in-place DVE add of the prefetched attn rows
                    V.tensor_tensor(
                        yflat, yflat, ar[:].rearrange("p q c -> p (q c)"),
                        op=AL.add,
                    )
                    st = nc.gpsimd.dma_start(
                        out[
                            b,
                            oc * 128 : (oc + 1) * 128,
                            ty0 * 256 : (ty0 + nty) * 256,
                        ],
                        yflat,
                    )
                    continue
                # attention add in the DMA datapath (CCE), then store fp16
                acc = nc.gpsimd.dma_start(
                    yflat,
                    attn_chw[
                        oc * 128 : (oc + 1) * 128, ty0 : ty0 + nty, :
                    ].rearrange("p q c -> p (q c)"),
                    accum_op=AL.add,
                )
                # cross-queue DRAM RAW: attn rows for this oc chunk are the
                # o2 writes of token half oc (n = 16c + q)
                for dep in o2_dmas[(b, oc)]:
                    add_dep_helper(
                        acc.ins, dep.ins, sync=True, reason="attn rows written"
                    )
                add_dep_helper(acc.ins, last.ins, sync=True, reason="y ready")
                st = nc.gpsimd.dma_start(
                    out[b, oc * 128 : (oc + 1) * 128, ty0 * 256 : (ty0 + nty) * 256],
                    yflat,
                )
                add_dep_helper(st.ins, acc.ins, sync=True, reason="acc done")

        # ---------------- schedule (emission order == engine queue order) ----
        # gpsimd queue: b0 xr DMAs first, then b1's, then conv acc/stores.
        # sync queue: ident32, b0 staging, weights, b1 staging, o2 writes.
        xpad_in(0, 0, from_dram=True)
        xpad_in(0, 1, from_dram=True)
        x_load(0)
        weights_in()
        front = qk_phase if use_qkv_bias else g_phase
        front(0)
        v_build(0, 0)
        x_load(1)
        xpad_in(1, 0)
        xpad_in(1, 1)
        softmax_phase(0)
        v_build(0, 1)
        front(1)
        v_build(1, 0)
        rest_phase(0)
        softmax_phase(1)
        v_build(1, 1)
        conv_phase(0, tiles=ALL_TILES[:2])
        rest_phase(1)
        conv_phase(0, tiles=ALL_TILES[2:])
        conv_phase(1, tiles=ALL_TILES[:3])
        conv_phase(1, tiles=[(1, 8, 4), (1, 12, 4)], late=True)

    nc.compile()
    return nc


def _prep_inputs(x, qkv_w, qkv_b, proj_w, proj_b, conv_w, conv_b):
    f = np.float32
    h = np.float16
    x = np.ascontiguousarray(x, dtype=f).reshape(B, C, N)
    qkv_w = np.asarray(qkv_w, dtype=f)
    qkv_b = np.asarray(qkv_b, dtype=f)
    proj_w = np.asarray(proj_w, dtype=f)
    proj_b = np.asarray(proj_b, dtype=f)
    conv_w = np.asarray(conv_w, dtype=f)
    conv_b = np.asarray(conv_b, dtype=f)

    # [Wq*s | Wk] transposed: [256 in, 512 out] (scale folded into Q side)
    wqk_t = np.ascontiguousarray(
        np.concatenate([(qkv_w[:C] * SCALE).T, qkv_w[C : 2 * C].T], axis=1), dtype=h
    )
    wv = np.ascontiguousarray(qkv_w[2 * C :], dtype=h)
    pw_t = np.ascontiguousarray(proj_w.T, dtype=h)

    # winograd along H: transform the vertical taps (ky), keep dx explicit.
    # Gw[p, dx, ic, oc] packed as gw[kc][ic(128), p, dx, oc]
    Gw = np.einsum("pk,oikd->pdio", G_WINO, conv_w.astype(np.float64)).astype(f)
    gw = np.ascontiguousarray(
        Gw.transpose(2, 0, 1, 3).reshape(CK, 128, 6, 3, C), dtype=h
    )

    bqk = np.ascontiguousarray(
        np.concatenate([qkv_b[:C] * SCALE, qkv_b[C : 2 * C]]), dtype=h
    )
    bv = np.ascontiguousarray(qkv_b[2 * C :], dtype=h)

    flags = dict(
        use_qkv_bias=bool(np.any(bqk)),
        use_v_bias=bool(np.any(bv)),
        use_proj_bias=bool(np.any(proj_b)),
        use_conv_bias=bool(np.any(conv_b)),
    )
    shared = {
        "wqk_t": wqk_t,
        "wv": wv,
        "pw_t": pw_t,
        "gw": gw,
        "ident32": np.eye(128, dtype=f),
    }
    if flags["use_qkv_bias"]:
        shared["bqk"] = bqk
    if flags["use_v_bias"]:
        shared["bv"] = bv
    if flags["use_proj_bias"]:
        shared["pb"] = np.asarray(proj_b, dtype=h)
    if flags["use_conv_bias"]:
        shared["cb"] = conv_b

    in_maps = []
    for core in range(N_CORES):
        m = dict(shared)
        m["x"] = np.ascontiguousarray(x[core * BL : (core + 1) * BL])
        in_maps.append(m)
    return in_maps, flags


def run(inputs, trace=False):
    in_maps, flags = _prep_inputs(**inputs)
    nc = build_program(**flags)
    res = run_bass_kernel_spmd(nc, in_maps, list(range(N_CORES)), trace=trace)
    out = np.concatenate(
        [
            res.results[i]["out"].astype(np.float32).reshape(BL, C, H, W)
            for i in range(N_CORES)
        ],
        axis=0,
    )
    return out, res


def kernel(**inputs):
    out, _ = run(inputs, trace=False)
    return out


# revision 38
# speedup vs baseline: 1.0291x; 1.0291x over previous
"""Trainium2 Bass kernel for nn_Attention_27719718929033 (v3).

Channel-attention block + 3x3 conv, data-parallel over batch across 8 cores.

v3 changes vs v2 (217.8us, and a latent xpad read-before-write race):
  * X^T tiles for the gram matmul come from xbar DMA-transposes of the fp16
    x16 copy (SBUF->SBUF on the ACT HWDGE queue) instead of 128 PE
    fp32r transpose matmuls + fp32 staging + PSUM drain copies.  Removes
    ~7us of PE work, ~10us of DVE/ScalarE drain work, and one full fp32
    HBM read of x.
  * attention-add runs inside the DMA datapath: the attn token-major
    rows are DMA-accumulated (CCE add) onto the winograd inverse-transform
    output in SBUF, then stored with one more DMA.  Removes the separate
    GpSimd/DVE add ops, the ar staging tiles, and the fp32 co tiles.
  * out is declared fp16 in DRAM and upcast on host (exact): halves the
    output write traffic.
  * xpad border memsets + interior DMAs get explicit dependency edges to
    the first DVE op of each v_build: the v2 flat-view reads raced with
    the strided border memsets (nondeterministic NaN / 2e-2 errors).
  * o2->attn DRAM writes stay on sync; acc-DMAs get explicit deps on the
    o2 writes they read (cross-queue DRAM RAW).

Math (per batch, X = x[b] in [C, N] fp16, N = 4096):
    G = X X^T (gram); logits = Wq_s G Wk^T; A = softmax_rows(logits)
    M = pw A Wv; out2 = X^T M^T  (token-major [N, C])
    reference reshape [N, C] -> [B, C, H, W] is a flat reinterpretation =>
    write out2 token-major to DRAM fp16, DMA-accumulate slices of it onto
    the conv output (channel-major read absorbs the reinterpretation).
    conv = F(4,3)-Winograd 3x3 along H; fp16 everywhere, fp32 PSUM.
"""
from contextlib import ExitStack

import numpy as np

import concourse.bacc as bacc
import concourse.mybir as mybir
import concourse.tile as tile
from concourse.bass_utils import run_bass_kernel_spmd
from concourse.tile_rust import add_dep_helper

N_CORES = 8
B, C, H, W = 16, 256, 64, 64
BL = B // N_CORES  # batches per core
N = H * W  # tokens
HP = H + 2  # padded height
WP = W + 2  # padded width
CK = C // 128  # channel chunks of 128
TT = N // 128  # token tiles of 128
TY = H // 4  # winograd tiles along H (4 output rows each)
NHB = H // 32  # conv h-blocks of 32 rows
SCALE = C ** (-0.5)

F32 = mybir.dt.float32
F32R = mybir.dt.float32r
F16 = mybir.dt.float16

# F(4,3) Winograd weight transform (Lavin), for correlation (= lax conv).
G_WINO = np.array(
    [
        [1 / 4, 0, 0],
        [-1 / 6, -1 / 6, -1 / 6],
        [-1 / 6, 1 / 6, -1 / 6],
        [1 / 24, 1 / 12, 1 / 6],
        [1 / 24, -1 / 12, 1 / 6],
        [0, 0, 1],
    ],
    dtype=np.float64,
)


def build_program(use_qkv_bias, use_v_bias, use_proj_bias, use_conv_bias):
    nc = bacc.Bacc(None, target_bir_lowering=False)
    AL = mybir.AluOpType

    x = nc.declare_dram_parameter("x", [BL, C, N], F32R, isOutput=False)
    wqk_t = nc.declare_dram_parameter("wqk_t", [C, 2 * C], F16, isOutput=False)
    wv = nc.declare_dram_parameter("wv", [C, C], F16, isOutput=False)
    pw_t = nc.declare_dram_parameter("pw_t", [C, C], F16, isOutput=False)
    # gw[kc][ic(128), p(6), dy(3), oc(256)] fp16 winograd-transformed conv w
    gw = nc.declare_dram_parameter("gw", [CK, 128, 6, 3, C], F16, isOutput=False)
    bqk = bv = pb = cb = None
    if use_qkv_bias:
        bqk = nc.declare_dram_parameter("bqk", [2 * C], F16, isOutput=False)
    if use_v_bias:
        bv = nc.declare_dram_parameter("bv", [C], F16, isOutput=False)
    if use_proj_bias:
        pb = nc.declare_dram_parameter("pb", [C], F16, isOutput=False)
    if use_conv_bias:
        cb = nc.declare_dram_parameter("cb", [C], F32, isOutput=False)
    ident32 = nc.declare_dram_parameter("ident32", [128, 128], F32R, isOutput=False)
    out = nc.declare_dram_parameter("out", [BL, C, N], F16, isOutput=True)

    attn_dram = nc.dram_tensor("attn_scratch", [BL, N, C], F16)

    with tile.TileContext(nc) as tc, ExitStack() as ctx:
        # --- persistent SBUF pools ---
        xs_pool = ctx.enter_context(tc.tile_pool(name="x16", bufs=1))
        xp_pool = ctx.enter_context(tc.tile_pool(name="xpad", bufs=4))
        v_pool = ctx.enter_context(tc.tile_pool(name="vwin", bufs=2))
        w_pool = ctx.enter_context(tc.tile_pool(name="weights", bufs=1))
        sm_pool = ctx.enter_context(tc.tile_pool(name="smx", bufs=2))
        attn_pool = ctx.enter_context(tc.tile_pool(name="attnmat", bufs=1))
        o2_pool = ctx.enter_context(tc.tile_pool(name="o2", bufs=3))
        m_pool = ctx.enter_context(tc.tile_pool(name="msb", bufs=3))
        it_pool = ctx.enter_context(tc.tile_pool(name="invtmp", bufs=1))
        ar_pool = ctx.enter_context(tc.tile_pool(name="attnrd", bufs=2))
        # PSUM: 2 (g/logits) + 2 (mm rotation) + 4 (conv point-pairs) = 8 banks
        at_ps_pool = ctx.enter_context(
            tc.tile_pool(name="atps", bufs=2, space="PSUM")
        )
        mm_ps_pool = ctx.enter_context(
            tc.tile_pool(name="mmps", bufs=2, space="PSUM")
        )
        cv_ps_pool = ctx.enter_context(
            tc.tile_pool(name="cvps", bufs=2, space="PSUM")
        )

        # --- weights to SBUF ---
        wqk_sb = w_pool.tile([128, CK, 2 * C], F16, tag="wqk")
        wv_sb = w_pool.tile([128, CK, C], F16, tag="wv")
        pw_sb = w_pool.tile([128, CK, C], F16, tag="pw")
        gw_sb = [
            w_pool.tile([128, 6, 3, C], F16, tag=f"gw{kc}", name=f"gw_sb{kc}")
            for kc in range(CK)
        ]

        def weights_in(part):
            # wqk right after the b0 staging (needed by T1 ~28us); the rest
            # (3MB of HBM) after the b1 staging to keep the head window clear
            if part == 0:
                for kc in range(CK):
                    nc.sync.dma_start(
                        wqk_sb[:, kc, :], wqk_t[kc * 128 : (kc + 1) * 128, :]
                    )
            else:
                for kc in range(CK):
                    nc.sync.dma_start(
                        wv_sb[:, kc, :], wv[kc * 128 : (kc + 1) * 128, :]
                    )
                    nc.sync.dma_start(
                        pw_sb[:, kc, :], pw_t[kc * 128 : (kc + 1) * 128, :]
                    )
                for kc in range(CK):
                    nc.sync.dma_start(gw_sb[kc][:], gw[kc])

        ones1 = None
        if use_qkv_bias or use_v_bias or use_proj_bias:
            ones1 = w_pool.tile([1, 128], F16, tag="ones")
            nc.gpsimd.memset(ones1[:], 1.0)
        bqk_sb = None
        if use_qkv_bias:
            bqk_sb = w_pool.tile([1, 2 * C], F16, tag="bqk")
            nc.sync.dma_start(bqk_sb[:], bqk[:].rearrange("c -> 1 c"))
        bv_sb = None
        if use_v_bias:
            bv_sb = w_pool.tile([128, CK], F16, tag="bv")
            for dc in range(CK):
                nc.sync.dma_start(
                    bv_sb[:, dc], bv[dc * 128 : (dc + 1) * 128].rearrange("p -> p 1")
                )
        pb_sb = None
        if use_proj_bias:
            pb_sb = w_pool.tile([1, C], F16, tag="pb")
            nc.sync.dma_start(pb_sb[:], pb[:].rearrange("c -> 1 c"))
        cb_sb = None
        if use_conv_bias:
            cb_sb = w_pool.tile([128, CK], F32, tag="cb")
            for oc in range(CK):
                nc.sync.dma_start(
                    cb_sb[:, oc], cb[oc * 128 : (oc + 1) * 128].rearrange("p -> p 1")
                )

        # --- input: fp32r staging chunks (sync HWDGE, line rate).  x16 via
        # ScalarE casts from staging; X^T tiles via PE fp32r transpose-mode
        # matmuls inside g_phase (xbar DMA-transposes act as global DMA
        # barriers in Tile and serialized the whole head).
        x16 = [
            [
                xs_pool.tile([128, N], F16, tag=f"x{b}{ck}", name=f"x16_{b}_{ck}")
                for ck in range(CK)
            ]
            for b in range(BL)
        ]
        ident32_sb = w_pool.tile([128, 128], F32R, tag="ident32")
        nc.sync.dma_start(ident32_sb[:], ident32[:])
        stage_pool = ctx.enter_context(tc.tile_pool(name="xstage", bufs=6))
        NSC = 1024  # staging chunk tokens
        stage = {}
        x16_casts = {}

        def x_load(b):
            for s in range(4):
                for ck in range(CK):
                    o = s * NSC
                    st = stage_pool.tile([128, NSC], F32R, tag="xstg")
                    nc.sync.dma_start(
                        st[:], x[b, ck * 128 : (ck + 1) * 128, o : o + NSC]
                    )
                    ci = nc.scalar.copy(
                        x16[b][ck][:, o : o + NSC], st[:].bitcast(F32)
                    )
                    x16_casts.setdefault((b, ck), []).append(ci)
                    stage[(b, ck, s)] = st

        def tok_window(b, ck, t):
            # stationary [128 chan, 128 tokens] fp16 (one contiguous free dim)
            return x16[b][ck][:, t * 128 : (t + 1) * 128]

        # --- xr: padded, row-deinterleaved fp16 x for the winograd V build ---
        # layout [128, 4, 18, WP]: image row h lives at [h % 4, h // 4 + 1];
        # top pad (h=-1) at [3][0], bottom pad (h=64) at [0][17].  Strided
        # casting DMAs from DRAM (gpsimd SWDGE): ~1.4us gen + ~1.2us
        # transfer each, and no engine time.
        xpad = {}
        xpad_deps = {}  # (b, kc) -> list of producing insts (for v_build deps)

        def xpad_in(b, kc, from_dram=False):
            xp = xp_pool.tile(
                [128, 4, 18, WP], F16, tag="xp", name=f"xpad{b}_{kc}"
            )
            # borders: pad cols 0/65 for all rows; pad rows at flat (r m)
            # indices [0][17] = 17 and [3][0] = 54 (stride 37).
            m1 = nc.gpsimd.memset(
                xp[:].rearrange("p r m w -> p (r m) w")[:, :, 0 :: WP - 1], 0.0
            )
            m2 = nc.gpsimd.memset(
                xp[:].rearrange("p r m w -> p (r m) w")[:, 17:55:37, :], 0.0
            )
            deps = [m1, m2]
            if from_dram:
                # b0: strided casting DMAs straight from DRAM (SWDGE) --
                # they run before x16 exists and overlap the staging reads
                xv = x[b, kc * 128 : (kc + 1) * 128, :].bitcast(F32).rearrange(
                    "p (h w) -> p h w", w=W
                )
                for r in range(4):
                    deps.append(
                        nc.gpsimd.dma_start(
                            xp[:, r, 1:17, 1 : W + 1], xv[:, r::4, :]
                        )
                    )
            else:
                # b1: plain SBUF->SBUF copies from x16 on the sync HWDGE
                # queue: no HBM traffic, no SWDGE-queue blockage ahead of
                # the conv acc DMAs
                xv = x16[b][kc][:].rearrange("p (h w) -> p h w", w=W)
                for r in range(4):
                    d = nc.sync.dma_start(
                        xp[:, r, 1:17, 1 : W + 1], xv[:, r::4, :]
                    )
                    for ci in x16_casts[(b, kc)]:
                        add_dep_helper(
                            d.ins, ci.ins, sync=True, reason="x16 cast done"
                        )
                    deps.append(d)
            xpad[(b, kc)] = xp
            xpad_deps[(b, kc)] = deps

        v_sb = {}

        def v_build(b, kc):
            # V_p = sum_q BT[p,q] d_q with d_q = deinterleaved xpad phase
            # reads (flat contiguous [128, 1056] -> DVE 2x TT / 4x TS modes).
            vt = v_pool.tile(
                [128, 6, TY, WP], F16, tag=f"v{b}", name=f"v_{b}_{kc}"
            )
            vtf = vt[:].rearrange("p s t w -> p (s t w)")
            xpf = xpad[(b, kc)][:].rearrange("p r m w -> p (r m w)")
            SEG = TY * WP  # 1056

            def d(q):
                # d(q)[ty] = padded row 4ty+q: [r=(q-1)%4][slot (q-1)//4+1..]
                r, s0 = (q - 1) % 4, (q - 1) // 4 + 1
                off = r * (18 * WP) + s0 * WP
                return xpf[:, off : off + SEG]

            def vs(j):
                return vtf[:, j * SEG : (j + 1) * SEG]

            V = nc.vector
            TS, TTo = V.tensor_scalar_mul, V.tensor_tensor
            # V0 = 4 d0 - 5 d2 + d4   (v2 as scratch)
            first = TS(vs(2), d(2), -5.0)
            for dep in xpad_deps[(b, kc)]:
                add_dep_helper(
                    first.ins, dep.ins, sync=True, reason="xpad ready"
                )
            TS(vs(0), d(0), 4.0)
            TTo(vs(0), vs(0), d(4), op=AL.add)
            TTo(vs(0), vs(0), vs(2), op=AL.add)
            # V5 = 4 d1 - 5 d3 + d5   (v1 as scratch)
            TS(vs(1), d(1), 4.0)
            TTo(vs(1), vs(1), d(5), op=AL.add)
            TS(vs(5), d(3), -5.0)
            TTo(vs(5), vs(5), vs(1), op=AL.add)
            # V1 = -4(d1 + d2) + (d3 + d4)   (v2 as scratch)
            TTo(vs(2), d(1), d(2), op=AL.add)
            TTo(vs(1), d(3), d(4), op=AL.add)
            TS(vs(2), vs(2), -4.0)
            TTo(vs(1), vs(1), vs(2), op=AL.add)
            # V2 = 4(d1 - d2) + (d4 - d3)    (v3 as scratch)
            TTo(vs(2), d(1), d(2), op=AL.subtract)
            TS(vs(2), vs(2), 4.0)
            TTo(vs(3), d(4), d(3), op=AL.subtract)
            TTo(vs(2), vs(2), vs(3), op=AL.add)
            # V3 = 2u + v, V4 = -2u + v with u = d3-d1, v = d4-d2
            TTo(vs(4), d(3), d(1), op=AL.subtract)
            TTo(vs(3), d(4), d(2), op=AL.subtract)
            TS(vs(4), vs(4), 2.0)
            TTo(vs(3), vs(3), vs(4), op=AL.add)
            TS(vs(4), vs(4), -2.0)
            TTo(vs(4), vs(3), vs(4), op=AL.add)
            v_sb[(b, kc)] = vt

        # ---------------- attention (gram path, fp16 operands) ----------------
        lg_pss = {}
        a_sbs = {}

        def qk_phase(b):
            # explicit fused [Q|K] + logits (only used when qkv bias nonzero)
            lg_ps = at_ps_pool.tile([128, CK, C], F32, tag="atps", name=f"lg_ps{b}")
            for t in range(TT):
                qk_ps = mm_ps_pool.tile([128, 2 * C], F32, tag="qkps")
                for kc in range(CK):
                    nc.tensor.matmul(
                        qk_ps[:],
                        tok_window(b, kc, t),
                        wqk_sb[:, kc, :],
                        start=(kc == 0),
                        stop=(kc == CK - 1 and not use_qkv_bias),
                    )
                if use_qkv_bias:
                    nc.tensor.matmul(
                        qk_ps[:], ones1[:], bqk_sb[:], start=False, stop=True
                    )
                qk_sb = sm_pool.tile([128, 2 * C], F16, tag="qksb")
                nc.vector.tensor_copy(qk_sb[:], qk_ps[:])

                for cc in range(CK):
                    mm = nc.tensor.matmul(
                        lg_ps[:, cc, :],
                        qk_sb[:, cc * 128 : (cc + 1) * 128],
                        qk_sb[:, C : 2 * C],
                        start=(t == 0 and cc == 0),
                        stop=(t == TT - 1),
                        skip_group_check=True,
                    )
                    if t == 0 and cc == 0:
                        lg_clear = mm
                    elif t == 0:
                        add_dep_helper(
                            mm.ins, lg_clear.ins, sync=False,
                            reason="after lg bank clear",
                        )
            lg_pss[b] = lg_ps

        def g_phase(b):
            # logits = Wq_s (X X^T) Wk^T; X^T tiles via fp32r transpose-mode
            # matmuls on the staging tiles (~73ns each), drained fp16
            g_ps = at_ps_pool.tile([128, CK, C], F32, tag="atps", name=f"g_ps{b}")
            g_clear = None
            for t2 in range(TT // 2):
                xt_ps = mm_ps_pool.tile([128, 2, C], F32, tag="qkps")
                tclear = None
                for j in range(2):
                    t = 2 * t2 + j
                    s, jj = divmod(t, NSC // 128)
                    for ck in range(CK):
                        mm = nc.tensor.matmul(
                            xt_ps[:, j, ck * 128 : (ck + 1) * 128].bitcast(F32R),
                            stage[(b, ck, s)][:, jj * 128 : (jj + 1) * 128],
                            ident32_sb[:],
                            is_transpose=True,
                            start=(j == 0 and ck == 0),
                            stop=(j == 1 and ck == CK - 1),
                            skip_group_check=True,
                        )
                        if j == 0 and ck == 0:
                            tclear = mm
                        else:
                            add_dep_helper(
                                mm.ins, tclear.ins, sync=False,
                                reason="after xt bank clear",
                            )
                xt_sb = sm_pool.tile([128, 2, C], F16, tag="qksb", bufs=4)
                if t2 % 2 == 0:
                    nc.vector.tensor_copy(xt_sb[:], xt_ps[:])
                else:
                    nc.scalar.copy(xt_sb[:], xt_ps[:])
                for j in range(2):
                    t = 2 * t2 + j
                    for cc in range(CK):
                        mm = nc.tensor.matmul(
                            g_ps[:, cc, :],
                            xt_sb[:, j, cc * 128 : (cc + 1) * 128],
                            xt_sb[:, j, :],
                            start=(t == 0 and cc == 0),
                            stop=(t == TT - 1),
                            skip_group_check=True,
                        )
                        if t == 0 and cc == 0:
                            g_clear = mm
                        elif t == 0:
                            add_dep_helper(
                                mm.ins, g_clear.ins, sync=False,
                                reason="after g bank clear",
                            )
            g_sb = attn_pool.tile([128, CK, C], F16, tag="g", name=f"g_sb{b}")
            nc.scalar.copy(g_sb[:, 0, :], g_ps[:, 0, :])
            nc.scalar.copy(g_sb[:, 1, :], g_ps[:, 1, :])

            # T1 = G Wk^T
            t1_ps = mm_ps_pool.tile([128, CK, C], F32, tag="qkps", name=f"t1_ps{b}")
            t1_clear = None
            for cpc in range(CK):
                for dc in range(CK):
                    mm = nc.tensor.matmul(
                        t1_ps[:, cpc, :],
                        g_sb[:, dc, cpc * 128 : (cpc + 1) * 128],
                        wqk_sb[:, dc, C : 2 * C],
                        start=(cpc == 0 and dc == 0),
                        stop=(dc == CK - 1),
                        skip_group_check=True,
                    )
                    if cpc == 0 and dc == 0:
                        t1_clear = mm
                    elif dc == 0:
                        add_dep_helper(
                            mm.ins, t1_clear.ins, sync=False,
                            reason="after t1 bank clear",
                        )
            t1_sb = attn_pool.tile([128, CK, C], F16, tag="t1", name=f"t1_sb{b}")
            nc.scalar.copy(t1_sb[:, 0, :], t1_ps[:, 0, :])
            nc.scalar.copy(t1_sb[:, 1, :], t1_ps[:, 1, :])

            # logits = Wq_s T1
            lg_ps = at_ps_pool.tile([128, CK, C], F32, tag="atps", name=f"glg_ps{b}")
            lg_clear = None
            for cc in range(CK):
                for kc in range(CK):
                    mm = nc.tensor.matmul(
                        lg_ps[:, cc, :],
                        wqk_sb[:, kc, cc * 128 : (cc + 1) * 128],
                        t1_sb[:, kc, :],
                        start=(cc == 0 and kc == 0),
                        stop=(kc == CK - 1),
                        skip_group_check=True,
                    )
                    if cc == 0 and kc == 0:
                        lg_clear = mm
                    elif kc == 0:
                        add_dep_helper(
                            mm.ins, lg_clear.ins, sync=False,
                            reason="after glg bank clear",
                        )
            lg_pss[b] = lg_ps

        def softmax_phase(b):
            lg_ps = lg_pss[b]
            a_sb = attn_pool.tile([128, CK, C], F16, tag="a", name=f"a_sb{b}")
            ex = sm_pool.tile([128, CK, C], F16, tag="ex")
            for cc in range(CK):
                nmx = sm_pool.tile([128, 1], F32, tag=f"nmx{cc}", name=f"nmx{b}_{cc}")
                nc.vector.reduce_max(
                    nmx[:], lg_ps[:, cc, :], axis=mybir.AxisListType.X, negate=True
                )
                sm = sm_pool.tile([128, 1], F32, tag=f"sm{cc}", name=f"sm{b}_{cc}")
                nc.scalar.activation(
                    ex[:, cc, :],
                    lg_ps[:, cc, :],
                    mybir.ActivationFunctionType.Exp,
                    bias=nmx[:],
                    scale=1.0,
                    accum_out=sm[:],
                )
                rs = sm_pool.tile([128, 1], F32, tag=f"rs{cc}", name=f"rs{b}_{cc}")
                nc.vector.reciprocal(rs[:], sm[:])
                nc.vector.tensor_scalar_mul(a_sb[:, cc, :], ex[:, cc, :], rs[:])
            a_sbs[b] = a_sb

        o2_dmas = {}  # (b, oc-half) -> list of o2 write DMAs

        def rest_phase(b):
            a_sb = a_sbs[b]
            # U = A^T P^T
            u_sb = attn_pool.tile([128, CK, C], F16, tag="u", name=f"u_sb{b}")
            u_ps = mm_ps_pool.tile([128, CK, C], F32, tag="qkps", name=f"u_ps{b}")
            for dc in range(CK):
                for cc in range(CK):
                    mm = nc.tensor.matmul(
                        u_ps[:, dc, :],
                        a_sb[:, cc, dc * 128 : (dc + 1) * 128],
                        pw_sb[:, cc, :],
                        start=(dc == 0 and cc == 0),
                        stop=(cc == CK - 1),
                        skip_group_check=True,
                    )
                    if dc == 0 and cc == 0:
                        u_clear = mm
                    elif cc == 0:
                        add_dep_helper(
                            mm.ins, u_clear.ins, sync=False,
                            reason="after u bank clear",
                        )
            nc.scalar.copy(u_sb[:, 0, :], u_ps[:, 0, :])
            nc.scalar.copy(u_sb[:, 1, :], u_ps[:, 1, :])

            # M^T = Wv^T U
            mt_sb = attn_pool.tile([128, CK, C], F16, tag="mt", name=f"mt_sb{b}")
            mt_ps = mm_ps_pool.tile([128, CK, C], F32, tag="qkps", name=f"mt_ps{b}")
            for cpc in range(CK):
                for dc in range(CK):
                    mm = nc.tensor.matmul(
                        mt_ps[:, cpc, :],
                        wv_sb[:, dc, cpc * 128 : (cpc + 1) * 128],
                        u_sb[:, dc, :],
                        start=(cpc == 0 and dc == 0),
                        stop=(dc == CK - 1),
                        skip_group_check=True,
                    )
                    if cpc == 0 and dc == 0:
                        mt_clear = mm
                    elif dc == 0:
                        add_dep_helper(
                            mm.ins, mt_clear.ins, sync=False,
                            reason="after mt bank clear",
                        )
            nc.scalar.copy(mt_sb[:, 0, :], mt_ps[:, 0, :])
            nc.scalar.copy(mt_sb[:, 1, :], mt_ps[:, 1, :])

            # r^T = bv^T U + pb
            use_r = use_v_bias or use_proj_bias
            r_sb = None
            if use_r:
                r_ps = mm_ps_pool.tile([1, C], F32, tag="qkps")
                started = False
                if use_v_bias:
                    for dc in range(CK):
                        nc.tensor.matmul(
                            r_ps[:],
                            bv_sb[:, dc],
                            u_sb[:, dc, :],
                            start=(dc == 0),
                            stop=(dc == CK - 1 and not use_proj_bias),
                        )
                    started = True
                if use_proj_bias:
                    nc.tensor.matmul(
                        r_ps[:],
                        ones1[0:1, 0:1],
                        pb_sb[:],
                        start=not started,
                        stop=True,
                    )
                r_sb = attn_pool.tile([1, C], F16, tag="r", name=f"r_sb{b}")
                nc.vector.tensor_copy(r_sb[:], r_ps[:])

            # out2[n, e] = sum_c' X[c', n] M^T[c', e] (+ 1 r^T), fp16 to DRAM
            # two token-tiles share one PSUM bank + one drain copy + one DMA
            o2_dmas[(b, 0)] = []
            o2_dmas[(b, 1)] = []
            for t in range(0, TT, 2):
                o_ps = mm_ps_pool.tile([128, 2, C], F32, tag="qkps")
                o_clear = None
                for j in range(2):
                    for kc in range(CK):
                        mm = nc.tensor.matmul(
                            o_ps[:, j, :],
                            tok_window(b, kc, t + j),
                            mt_sb[:, kc, :],
                            start=(j == 0 and kc == 0),
                            stop=(kc == CK - 1 and not use_r),
                            skip_group_check=True,
                        )
                        if j == 0 and kc == 0:
                            o_clear = mm
                        elif kc == 0:
                            add_dep_helper(
                                mm.ins, o_clear.ins, sync=False,
                                reason="after o2 bank clear",
                            )
                    if use_r:
                        nc.tensor.matmul(
                            o_ps[:, j, :], ones1[:], r_sb[:], start=False, stop=True
                        )
                o_sb = o2_pool.tile([128, 2, C], F16, tag="o2sb")
                nc.scalar.copy(o_sb[:], o_ps[:])
                dma = nc.sync.dma_start(
                    attn_dram[b, t * 128 : (t + 2) * 128, :].rearrange(
                        "(a p) c -> p a c", p=128
                    ),
                    o_sb[:],
                )
                o2_dmas[(b, t // 16)].append(dma)

        # ---------------- conv: winograd point matmuls + inverse ----------------
        # tiles are (oc, ty0, nty): nty*4 output rows each.  The final tiles
        # run as halves (nty=4) to shorten the drain->inverse->acc->store
        # tail after the last matmul.
        ALL_TILES = [(oc, hb * 8, 8) for oc in range(CK) for hb in range(NHB)]

        def conv_phase(b, tiles=None, late=False):
            attn_chw = attn_dram[b].rearrange("(p q) c -> p q c", p=C)
            for oc, ty0, nty in tiles if tiles is not None else ALL_TILES:
                FD = nty * 64  # moving free dim per point matmul
                ar = None
                if late:
                    # prefetch the attn rows; the add runs on DVE so the
                    # tail skips the ~5us accumulate-DMA latency
                    ar = ar_pool.tile([128, nty, C], F16, tag="ar")
                    ard = nc.sync.dma_start(
                        ar[:],
                        attn_chw[oc * 128 : (oc + 1) * 128, ty0 : ty0 + nty, :],
                    )
                    for dep in o2_dmas[(b, oc)]:
                        add_dep_helper(
                            ard.ins, dep.ins, sync=True,
                            reason="attn rows written",
                        )
                m_sb = m_pool.tile(
                    [128, 6, FD], F16, tag="m", name=f"m_{b}_{oc}_{ty0}"
                )
                for grp in range(3):  # point pairs (0,1),(2,3),(4,5)
                    mp = cv_ps_pool.tile([128, 2, FD], F32, tag="cvps")
                    for pp in range(2):
                        p = grp * 2 + pp
                        for dx in range(3):
                            for kc in range(CK):
                                nc.tensor.matmul(
                                    mp[:, pp, :],
                                    gw_sb[kc][
                                        :, p, dx, oc * 128 : (oc + 1) * 128
                                    ],
                                    v_sb[(b, kc)][
                                        :, p, ty0 : ty0 + nty, dx : dx + W
                                    ],
                                    start=(dx == 0 and kc == 0),
                                    stop=(dx == 2 and kc == CK - 1),
                                )
                    if use_conv_bias and grp == 0:
                        # fold conv bias into m0 only: A^T row sums give
                        # y_i += cb exactly for i=0 and 0 elsewhere
                        nc.scalar.activation(
                            m_sb[:, 0, :], mp[:, 0, :],
                            mybir.ActivationFunctionType.Copy,
                            bias=cb_sb[:, oc], scale=1.0,
                        )
                        nc.scalar.copy(m_sb[:, 1, :], mp[:, 1, :])
                    else:
                        nc.scalar.copy(m_sb[:, 2 * grp : 2 * grp + 2, :], mp[:])

                # A^T inverse transform (DVE); y-phase outputs land
                # interleaved (rows i::4) in the dead m1..m4 slots so the
                # flat [128, 2048] view is row-major [32, 64]
                it = it_pool.tile([128, 6, FD], F16, tag="it")
                m_ = [m_sb[:, p, :] for p in range(6)]
                ia, ib, ic_, id_, ie, it3 = (it[:, j] for j in range(6))
                y16 = m_sb[:, 1:5, :].rearrange(
                    "p a f -> p (a f)"
                ).rearrange("p (h w) -> p h w", w=W)

                def rv(ap):
                    return ap.rearrange("p (ty w) -> p ty w", w=W)

                V = nc.vector
                V.tensor_tensor(ia, m_[1], m_[2], op=AL.subtract)
                V.tensor_tensor(ib, m_[3], m_[4], op=AL.subtract)
                V.tensor_tensor(ic_, m_[1], m_[2], op=AL.add)
                V.tensor_tensor(id_, m_[3], m_[4], op=AL.add)
                # m1..m4 are dead from here; their slots hold y0..y3 phases
                V.tensor_tensor(ie, ic_, id_, op=AL.add)
                V.tensor_tensor(y16[:, 0::4, :], rv(ie), rv(m_[0]), op=AL.add)
                V.scalar_tensor_tensor(it3, ib, 8.0, m_[5], AL.mult, AL.add)
                V.tensor_tensor(y16[:, 3::4, :], rv(it3), rv(ia), op=AL.add)
                V.scalar_tensor_tensor(
                    y16[:, 1::4, :], rv(ib), 2.0, rv(ia), AL.mult, AL.add
                )
                last = V.scalar_tensor_tensor(
                    y16[:, 2::4, :], rv(id_), 4.0, rv(ic_), AL.mult, AL.add
                )
                yflat = m_sb[:, 1:5, :].rearrange("p a f -> p (a f)")
                if late:
                    # in-place DVE add of the prefetched attn rows
                    V.tensor_tensor(
                        yflat, yflat, ar[:].rearrange("p q c -> p (q c)"),
                        op=AL.add,
                    )
                    st = nc.gpsimd.dma_start(
                        out[
                            b,
                            oc * 128 : (oc + 1) * 128,
                            ty0 * 256 : (ty0 + nty) * 256,
                        ],
                        yflat,
                    )
                    continue
                # attention add in the DMA datapath (CCE), then store fp16
                acc = nc.gpsimd.dma_start(
                    yflat,
                    attn_chw[
                        oc * 128 : (oc + 1) * 128, ty0 : ty0 + nty, :
                    ].rearrange("p q c -> p (q c)"),
                    accum_op=AL.add,
                )
                # cross-queue DRAM RAW: attn rows for this oc chunk are the
                # o2 writes of token half oc (n = 16c + q)
                for dep in o2_dmas[(b, oc)]:
                    add_dep_helper(
                        acc.ins, dep.ins, sync=True, reason="attn rows written"
                    )
                add_dep_helper(acc.ins, last.ins, sync=True, reason="y ready")
                st = nc.gpsimd.dma_start(
                    out[b, oc * 128 : (oc + 1) * 128, ty0 * 256 : (ty0 + nty) * 256],
                    yflat,
                )
                add_dep_helper(st.ins, acc.ins, sync=True, reason="acc done")

        # ---------------- schedule (emission order == engine queue order) ----
        # gpsimd queue: b0 xr DMAs first, then b1's, then conv acc/stores.
        # sync queue: ident32, b0 staging, weights, b1 staging, o2 writes.
        xpad_in(0, 0, from_dram=True)
        x_load(0)
        weights_in(0)
        front = qk_phase if use_qkv_bias else g_phase
        front(0)
        v_build(0, 0)
        x_load(1)
        weights_in(1)
        xpad_in(0, 1, from_dram=True)
        xpad_in(1, 0, from_dram=True)
        xpad_in(1, 1, from_dram=True)
        softmax_phase(0)
        v_build(0, 1)
        front(1)
        v_build(1, 0)
        rest_phase(0)
        softmax_phase(1)
        v_build(1, 1)
        conv_phase(0, tiles=ALL_TILES[:2])
        rest_phase(1)
        conv_phase(0, tiles=ALL_TILES[2:])
        conv_phase(1, tiles=ALL_TILES[:3])
        conv_phase(1, tiles=[(1, 8, 4), (1, 12, 4)], late=True)

    nc.compile()
    return nc


def _prep_inputs(x, qkv_w, qkv_b, proj_w, proj_b, conv_w, conv_b):
    f = np.float32
    h = np.float16
    x = np.ascontiguousarray(x, dtype=f).reshape(B, C, N)
    qkv_w = np.asarray(qkv_w, dtype=f)
    qkv_b = np.asarray(qkv_b, dtype=f)
    proj_w = np.asarray(proj_w, dtype=f)
    proj_b = np.asarray(proj_b, dtype=f)
    conv_w = np.asarray(conv_w, dtype=f)
    conv_b = np.asarray(conv_b, dtype=f)

    # [Wq*s | Wk] transposed: [256 in, 512 out] (scale folded into Q side)
    wqk_t = np.ascontiguousarray(
        np.concatenate([(qkv_w[:C] * SCALE).T, qkv_w[C : 2 * C].T], axis=1), dtype=h
    )
    wv = np.ascontiguousarray(qkv_w[2 * C :], dtype=h)
    pw_t = np.ascontiguousarray(proj_w.T, dtype=h)

    # winograd along H: transform the vertical taps (ky), keep dx explicit.
    # Gw[p, dx, ic, oc] packed as gw[kc][ic(128), p, dx, oc]
    Gw = np.einsum("pk,oikd->pdio", G_WINO, conv_w.astype(np.float64)).astype(f)
    gw = np.ascontiguousarray(
        Gw.transpose(2, 0, 1, 3).reshape(CK, 128, 6, 3, C), dtype=h
    )

    bqk = np.ascontiguousarray(
        np.concatenate([qkv_b[:C] * SCALE, qkv_b[C : 2 * C]]), dtype=h
    )
    bv = np.ascontiguousarray(qkv_b[2 * C :], dtype=h)

    flags = dict(
        use_qkv_bias=bool(np.any(bqk)),
        use_v_bias=bool(np.any(bv)),
        use_proj_bias=bool(np.any(proj_b)),
        use_conv_bias=bool(np.any(conv_b)),
    )
    shared = {
        "wqk_t": wqk_t,
        "wv": wv,
        "pw_t": pw_t,
        "gw": gw,
        "ident32": np.eye(128, dtype=f),
    }
    if flags["use_qkv_bias"]:
        shared["bqk"] = bqk
    if flags["use_v_bias"]:
        shared["bv"] = bv
    if flags["use_proj_bias"]:
        shared["pb"] = np.asarray(proj_b, dtype=h)
    if flags["use_conv_bias"]:
        shared["cb"] = conv_b

    in_maps = []
    for core in range(N_CORES):
        m = dict(shared)
        m["x"] = np.ascontiguousarray(x[core * BL : (core + 1) * BL])
        in_maps.append(m)
    return in_maps, flags


def run(inputs, trace=False):
    in_maps, flags = _prep_inputs(**inputs)
    nc = build_program(**flags)
    res = run_bass_kernel_spmd(nc, in_maps, list(range(N_CORES)), trace=trace)
    out = np.concatenate(
        [
            res.results[i]["out"].astype(np.float32).reshape(BL, C, H, W)
            for i in range(N_CORES)
        ],
        axis=0,
    )
    return out, res


def kernel(**inputs):
    out, _ = run(inputs, trace=False)
    return out
